# revision 10
# baseline (speedup 1.0000x reference)
# Trainium2 Bass kernel for nn_FDM_3899830304921 (feature-map cosine-sim
# dual-softmax transport), data-parallel over batch on 8 NeuronCores.
#
# v4: all transposes ride the DMA xbar (bf16) instead of the PE; the PE
# runs only matmuls (gram + the two output GEMMs). E^T comes from a DMA
# transpose of the bf16 exp output, so the colsum arrives free on the
# shift-2 accumulator and the old colsum matmul block is gone. fp8
# conversion copies live on GPSIMD; DVE keeps the quantizes (with exact
# S1/S2 accumulators), the shifted-E2 pass, and the output scaling.
#
# Math per batch (c=512, n=m=784 padded to 896 where noted):
#   f1b = bf16(f1) [c,n]  (+S1[c] accum)     f2b = bf16(f2) (+S2[c])
#   f1bT = dmaT(f1b) [n,c]; ssq1[n] (ACT square accum); r1=rsqrt(ssq1)
#   f2bT = dmaT(f2b) [m,c]; ssq2[m];          r2=rsqrt(ssq2)
#   f1q = fp8(f1b) (GPSIMD); f2n = fp8(-16*r2[m]*f2b) (DVE)
#   f1T = fp8(f1bT) (GPSIMD); f2T = fp8(f2bT)
#   G = f1q^T @ f2n (DR);  E1 = exp(G*r1/16) bf16  (+rowsum rs)
#   Ep1 = fp8(E1-1) (GPSIMD);  E2b = dmaT(E1) [m,n]
#   Ep2 = fp8(E2b-1) (DVE, accum -> colsum cs-784)
#   o2 = (f1T^T @ Ep1 + S1) * (.001/cs);  o1 = (f2T^T @ Ep2 + S2) * (.001/rs)
import sys

if "/opt/trn_rl_repo" not in sys.path:
    sys.path.insert(0, "/opt/trn_rl_repo")

import numpy as np

B_TOTAL = 32
B_PER_CORE = 4
N_CORES = 8
C = 512
N = 784   # 28*28, both spatial dims
NP = 896  # padded to 7*128 for the DMA xbar transpose
FACTOR = 0.001
RSQRT_SEED = 0.044194173824159216  # 1/sqrt(512)

HALVES = [(0, 512), (512, 272)]

_BUILT = {}


def _build(nbatch, enable_asserts=False):
    key = (nbatch, enable_asserts)
    if key in _BUILT:
        return _BUILT[key]

    import concourse.bass as bass
    import concourse.tile as tile
    from concourse import bacc, mybir

    f32 = mybir.dt.float32
    f8 = mybir.dt.float8e4
    bf16 = mybir.dt.bfloat16
    AF = mybir.ActivationFunctionType
    ALU = mybir.AluOpType
    DR = mybir.MatmulPerfMode.DoubleRow

    nc = bacc.Bacc("TRN2", target_bir_lowering=False, debug=False,
                   enable_asserts=enable_asserts, num_devices=N_CORES)
    fm1 = nc.dram_tensor("fm1", [nbatch, C, N], f32, kind="ExternalInput").ap()
    fm2 = nc.dram_tensor("fm2", [nbatch, C, N], f32, kind="ExternalInput").ap()
    o1 = nc.dram_tensor("o1", [nbatch, C, N], f32, kind="ExternalOutput").ap()
    o2 = nc.dram_tensor("o2", [nbatch, C, N], f32, kind="ExternalOutput").ap()

    with tile.TileContext(nc) as tc:
        with (
            tc.tile_pool(name="sb", bufs=2) as sb,
            tc.tile_pool(name="ps", bufs=2, space="PSUM") as ps,
            tc.tile_pool(name="dr", bufs=2, space="DRAM") as dram,
        ):
            def newton8(u):
                """[128, 8] f32 sums-of-squares -> rsqrt via 3 Newton iters."""
                yt = sb.tile([128, 8], f32, tag="nwt_y", bufs=2)
                ya = sb.tile([128, 8], f32, tag="nwt_a", bufs=2)
                nc.vector.memset(yt[:], RSQRT_SEED)
                for _ in range(3):
                    nc.vector.tensor_tensor(out=ya[:], in0=yt[:], in1=yt[:],
                                            op=ALU.mult)
                    nc.vector.tensor_tensor(out=ya[:], in0=ya[:], in1=u,
                                            op=ALU.mult)
                    nc.vector.tensor_scalar(
                        out=ya[:], in0=ya[:], scalar1=-0.5, scalar2=1.5,
                        op0=ALU.mult, op1=ALU.add)
                    nc.vector.tensor_tensor(out=yt[:], in0=yt[:], in1=ya[:],
                                            op=ALU.mult)
                return yt

            def row_bcast(colsb, tag):
                """[128, >=8] bf16 cols (col t = vals for t*128+p) ->
                [128, NP] bf16 broadcast tile, via dmaT + gather + bcast."""
                rT = sb.tile([128, 128], bf16, tag=tag + "_T", bufs=2)
                nc.sync.dma_start_transpose(out=rT[:], in_=colsb[:, 0:128])
                d = dram.tile([1, NP], bf16, tag=tag + "_d", bufs=2)
                nc.sync.dma_start(out=d[:].rearrange("a (t x) -> a t x", t=7),
                                  in_=rT[0:7, :])
                dap = d[:]
                srcap = bass.AP(tensor=dap.tensor, offset=dap.offset,
                                ap=[[0, 128]] + list(dap.ap))
                out = sb.tile([128, NP], bf16, tag=tag + "_B", bufs=2)
                nc.sync.dma_start(
                    out=out[:].rearrange("p (a x) -> p a x", a=1), in_=srcap)
                return out

            def load(b):
                f1c = sb.tile([128, 4, N], f32, tag="f1_32", bufs=2)
                f2c = sb.tile([128, 4, N], f32, tag="f2_32", bufs=2)
                for j in range(4):
                    csl = slice(j * 128, (j + 1) * 128)
                    nc.sync.dma_start(out=f1c[:, j, :], in_=fm1[b, csl, :])
                    nc.sync.dma_start(out=f2c[:, j, :], in_=fm2[b, csl, :])
                return f1c, f2c

            def prep(b, loaded):
                f1c, f2c = loaded
                f1b = sb.tile([128, 4, NP], bf16, tag="f1b", bufs=2)
                f2b = sb.tile([128, 4, NP], bf16, tag="f2b", bufs=2)
                if b < 2:  # zero the n-pad once per pool buffer
                    nc.gpsimd.memset(f1b[:, :, N:], 0)
                    nc.gpsimd.memset(f2b[:, :, N:], 0)
                s1 = sb.tile([128, 4], f32, tag="s1", bufs=2)
                s2 = sb.tile([128, 4], f32, tag="s2", bufs=2)
                with nc.allow_low_precision(reason="bf16 staging"):
                    for j in range(4):
                        nc.vector.tensor_scalar(
                            out=f1b[:, j, :N], in0=f1c[:, j, :],
                            scalar1=1.0, scalar2=0.0, op0=ALU.mult,
                            op1=ALU.add, accum_out=s1[:, j:j + 1])
                    for j in range(4):
                        nc.vector.tensor_scalar(
                            out=f2b[:, j, :N], in0=f2c[:, j, :],
                            scalar1=1.0, scalar2=0.0, op0=ALU.mult,
                            op1=ALU.add, accum_out=s2[:, j:j + 1])

                # transposed bf16 copies via the DMA xbar
                f1bT = sb.tile([128, 7, C], bf16, tag="f1bT", bufs=1)
                f2bT = sb.tile([128, 7, C], bf16, tag="f2bT", bufs=1)
                for j in range(4):
                    nc.sync.dma_start_transpose(
                        out=f1bT[:, 0:7, j * 128:(j + 1) * 128],
                        in_=f1b[:, j, :])
                for j in range(4):
                    nc.sync.dma_start_transpose(
                        out=f2bT[:, 0:7, j * 128:(j + 1) * 128],
                        in_=f2b[:, j, :])

                # fp8 copies
                f1q = sb.tile([128, 4, NP], f8, tag="f1q", bufs=2)
                nc.gpsimd.tensor_copy(out=f1q[:].rearrange("p a x -> p (a x)"),
                                      in_=f1b[:].rearrange("p a x -> p (a x)"))
                f1T = sb.tile([128, 7, C], f8, tag="f1T", bufs=2)
                nc.gpsimd.tensor_copy(out=f1T[:].rearrange("p a x -> p (a x)"),
                                      in_=f1bT[:].rearrange("p a x -> p (a x)"))
                f2T = sb.tile([128, 7, C], f8, tag="f2T", bufs=2)
                nc.gpsimd.tensor_copy(out=f2T[:].rearrange("p a x -> p (a x)"),
                                      in_=f2bT[:].rearrange("p a x -> p (a x)"))

                # sums of squares -> rsqrt
                ssq1 = sb.tile([128, 8], f32, tag="ssq1", bufs=2)
                ssq2 = sb.tile([128, 8], f32, tag="ssq2", bufs=2)
                nc.vector.memset(ssq1[:], 1.0)
                nc.vector.memset(ssq2[:], 1.0)
                junk = sb.tile([128, C], bf16, tag="junk", bufs=2)
                for t in range(7):
                    nc.scalar.activation(
                        out=junk[:], in_=f1bT[:, t, :], func=AF.Square,
                        accum_out=ssq1[:, t:t + 1])
                for t in range(7):
                    nc.scalar.activation(
                        out=junk[:], in_=f2bT[:, t, :], func=AF.Square,
                        accum_out=ssq2[:, t:t + 1])

                y1 = newton8(ssq1[:])
                r1s = sb.tile([128, 8], f32, tag="r1s", bufs=2)
                nc.vector.tensor_scalar(out=r1s[:], in0=y1[:], scalar1=0.0625,
                                        scalar2=None, op0=ALU.mult)
                y2 = newton8(ssq2[:])
                r2cb = sb.tile([128, 128], bf16, tag="r2cb", bufs=2)
                if b < 2:
                    nc.gpsimd.memset(r2cb[:, 8:], 0)
                with nc.allow_low_precision(reason="bf16 r2 cols"):
                    nc.vector.tensor_scalar(out=r2cb[:, 0:8], in0=y2[:],
                                            scalar1=1.0, scalar2=None,
                                            op0=ALU.mult)
                r2B = row_bcast(r2cb, "r2")

                f2n = sb.tile([128, 4, NP], f8, tag="f2n", bufs=2)
                with nc.allow_low_precision(reason="fp8 scaled f2"):
                    for j in range(4):
                        nc.vector.scalar_tensor_tensor(
                            out=f2n[:, j, :], in0=f2b[:, j, :], scalar=-16.0,
                            in1=r2B[:, :], op0=ALU.mult, op1=ALU.mult)
                return f1q, f2n, f1T, f2T, r1s, s1, s2

            def comp(b, state):
                f1q, f2n, f1T, f2T, r1s, s1, s2 = state

                # gram + exp + shift1 + E-transpose, per n-tile
                E1 = sb.tile([128, 7, NP], bf16, tag="E1", bufs=1)
                Ep1 = sb.tile([128, 7, N], f8, tag="Ep1", bufs=2)
                E2b = sb.tile([128, 7, NP], bf16, tag="E2b", bufs=1)
                rsc = sb.tile([128, 8], f32, tag="rsc", bufs=2)
                if b < 1:  # n-pad cols, read (as junk) by the E dma transpose
                    nc.gpsimd.memset(E1[:, :, N:], 0)
                nc.vector.memset(rsc[:], 1.0)
                for t in range(7):
                    G = ps.tile([128, 1024], f32, tag="G", bufs=2)
                    nsl = slice(t * 128, (t + 1) * 128)
                    for k in range(2):
                        for hoff, hsz in HALVES:
                            nc.tensor.matmul(
                                G[:, hoff:hoff + hsz],
                                f1q[:, 2 * k:2 * k + 2, nsl],
                                f2n[:, 2 * k:2 * k + 2, hoff:hoff + hsz],
                                start=(k == 0), stop=(k == 1), perf_mode=DR)
                    nc.scalar.activation(
                        out=E1[:, t, :N], in_=G[:, :N], func=AF.Exp,
                        scale=r1s[:, t:t + 1], accum_out=rsc[:, t:t + 1])
                    with nc.allow_low_precision(reason="fp8 shifted E1"):
                        nc.gpsimd.tensor_scalar(
                            out=Ep1[:, t, :], in0=E1[:, t, :N], scalar1=-1.0,
                            scalar2=None, op0=ALU.add)
                    nc.sync.dma_start_transpose(
                        out=E2b[:, 0:7, t * 128:(t + 1) * 128],
                        in_=E1[:, t, :])

                # rowsum -> rrB = .001/rs broadcast
                rrf = sb.tile([128, 8], f32, tag="rrf", bufs=2)
                nc.vector.reciprocal(rrf[:], rsc[:])
                rrcb = sb.tile([128, 128], bf16, tag="rrcb", bufs=2)
                if b < 2:
                    nc.gpsimd.memset(rrcb[:, 8:], 0)
                with nc.allow_low_precision(reason="bf16 scale rows"):
                    nc.vector.tensor_scalar(out=rrcb[:, 0:8], in0=rrf[:],
                                            scalar1=FACTOR, scalar2=None,
                                            op0=ALU.mult)
                rrB = row_bcast(rrcb, "rr")

                # shift2 with free colsum accum
                Ep2 = sb.tile([128, 7, NP], f8, tag="Ep2", bufs=2)
                csc = sb.tile([128, 8], f32, tag="csc", bufs=2)
                nc.vector.memset(csc[:], 1.0)
                with nc.allow_low_precision(reason="fp8 shifted E2"):
                    for u in range(7):
                        nc.vector.tensor_scalar(
                            out=Ep2[:, u, :], in0=E2b[:, u, :], scalar1=-1.0,
                            scalar2=0.0, op0=ALU.add, op1=ALU.add,
                            accum_out=csc[:, u:u + 1])
                rcf = sb.tile([128, 8], f32, tag="rcf", bufs=2)
                nc.vector.tensor_scalar(out=rcf[:], in0=csc[:], scalar1=1.0,
                                        scalar2=float(N), op0=ALU.mult,
                                        op1=ALU.add)
                nc.vector.reciprocal(rcf[:], rcf[:])
                rccb = sb.tile([128, 128], bf16, tag="rccb", bufs=2)
                if b < 2:
                    nc.gpsimd.memset(rccb[:, 8:], 0)
                with nc.allow_low_precision(reason="bf16 scale rows"):
                    nc.vector.tensor_scalar(out=rccb[:, 0:8], in0=rcf[:],
                                            scalar1=FACTOR, scalar2=None,
                                            op0=ALU.mult)
                rcB = row_bcast(rccb, "rc")

                def out_mm(dst, statT, mov, scol, sclB):
                    """dst[b, c, :] = (statT^T @ mov + scol) * sclB."""
                    for ci in range(4):
                        csl = slice(ci * 128, (ci + 1) * 128)
                        P = ps.tile([128, 1024], f32, tag="P", bufs=2)
                        for hoff, hsz in HALVES:
                            for u in range(3):
                                nc.tensor.matmul(
                                    P[:, hoff:hoff + hsz],
                                    statT[:, 2 * u:2 * u + 2, csl],
                                    mov[:, 2 * u:2 * u + 2, hoff:hoff + hsz],
                                    start=(u == 0), stop=False, perf_mode=DR)
                            nc.tensor.matmul(
                                P[:, hoff:hoff + hsz],
                                statT[:, 6, csl],
                                mov[:, 6, hoff:hoff + hsz],
                                start=False, stop=True)
                        O = sb.tile([128, N], f32, tag="O", bufs=3)
                        nc.vector.scalar_tensor_tensor(
                            out=O[:], in0=P[:, :N], scalar=scol[:, ci:ci + 1],
                            in1=sclB[:, :N], op0=ALU.add, op1=ALU.mult)
                        nc.sync.dma_start(out=dst[b, csl, :], in_=O[:])

                out_mm(o2, f1T, Ep1, s1, rcB)
                out_mm(o1, f2T, Ep2, s2, rrB)

            # loads run two batches ahead so prep(b+1)'s DVE work never
            # heads the queue waiting on an in-flight DMA
            loads = [load(0)]
            if nbatch > 1:
                loads.append(load(1))
            state = prep(0, loads[0])
            for b in range(nbatch):
                if b + 2 < nbatch:
                    loads.append(load(b + 2))
                nstate = prep(b + 1, loads[b + 1]) if b + 1 < nbatch else None
                comp(b, state)
                state = nstate

    nc.compile()
    _BUILT[key] = nc
    return nc


def _run(fm1, fm2, trace=False):
    from concourse.bass_utils import run_bass_kernel_spmd

    fm1 = np.ascontiguousarray(np.asarray(fm1, np.float32).reshape(B_TOTAL, C, N))
    fm2 = np.ascontiguousarray(np.asarray(fm2, np.float32).reshape(B_TOTAL, C, N))
    nc = _build(B_PER_CORE)
    f1s = fm1.reshape(N_CORES, B_PER_CORE, C, N)
    f2s = fm2.reshape(N_CORES, B_PER_CORE, C, N)
    in_maps = [
        {"fm1": np.ascontiguousarray(f1s[i]), "fm2": np.ascontiguousarray(f2s[i])}
        for i in range(N_CORES)
    ]
    res = run_bass_kernel_spmd(nc, in_maps, core_ids=list(range(N_CORES)),
                               trace=trace)
    out1 = np.concatenate([res.results[i]["o1"] for i in range(N_CORES)], axis=0)
    out2 = np.concatenate([res.results[i]["o2"] for i in range(N_CORES)], axis=0)
    out1 = out1.reshape(B_TOTAL, C, 28, 28).astype(np.float32)
    out2 = out2.reshape(B_TOTAL, C, 28, 28).astype(np.float32)
    return (out1, out2), res


def kernel(fm1, fm2):
    (out1, out2), _ = _run(fm1, fm2)
    return out1, out2


# revision 12
# speedup vs baseline: 1.4940x; 1.4940x over previous
# Trainium2 Bass kernel for nn_FDM_3899830304921 (feature-map cosine-sim
# dual-softmax transport), data-parallel over batch on 8 NeuronCores.
#
# v5: PE runs only matmuls (fp8-DR gram + two fp8-DR output GEMMs + a tiny
# ones-row colsum GEMM). All transposes ride the DMA xbar in bf16. The two
# E-shift passes run on DVE in bf16 at 4x mode; GPSIMD-issued cast-DMAs
# convert bf16->fp8 (f1q/f1T/f2T/Ep1/Ep2) so no engine pays the 1x
# fp8-output tax. f32->bf16 staging quantizes live on ACT and carry the
# exact S1/S2 accumulators. Software-pipelined two batches deep so every
# engine queue always has ready work at its head.
#
# Math per batch (c=512, n=m=784, padded to 896 for the xbar):
#   f1b=bf16(f1)(+S1), f2b=bf16(f2)(+S2)      [ACT copy+accum]
#   f1bT=dmaT(f1b), f2bT=dmaT(f2b)            [DMA xbar]
#   ssq1[n], ssq2[m] -> r=rsqrt (Newton, DVE)
#   f1q=fp8(f1b) [cast-DMA]; f2n=fp8(-16*r2[m]*f2b) [DVE STT]
#   f1T=fp8(f1bT), f2T=fp8(f2bT)              [cast-DMA]
#   G=f1q^T@f2n (DR); E1=exp(G*r1/16) (+rowsum rs)
#   Ep1b=E1-1 (DVE 4x) -> Ep1 fp8 [cast-DMA]; E2b=dmaT(E1)
#   Ep2b=E2b-1 (DVE 4x) -> Ep2 fp8 [cast-DMA]
#   cs[m] = ones^T @ Ep1 (PE) + N
#   o2=(f1T^T@Ep1+S1)*(.001/cs); o1=(f2T^T@Ep2+S2)*(.001/rs)
import sys

if "/opt/trn_rl_repo" not in sys.path:
    sys.path.insert(0, "/opt/trn_rl_repo")

import numpy as np

B_TOTAL = 32
B_PER_CORE = 4
N_CORES = 8
C = 512
N = 784   # 28*28, both spatial dims
NP = 896  # padded to 7*128 for the DMA xbar transpose
FACTOR = 0.001
RSQRT_SEED = 0.044194173824159216  # 1/sqrt(512)

HALVES = [(0, 512), (512, 272)]

_BUILT = {}


def _build(nbatch, enable_asserts=False):
    key = (nbatch, enable_asserts)
    if key in _BUILT:
        return _BUILT[key]

    import concourse.bass as bass
    import concourse.tile as tile
    from concourse import bacc, mybir

    f32 = mybir.dt.float32
    f32r = mybir.dt.float32r
    f8 = mybir.dt.float8e4
    bf16 = mybir.dt.bfloat16
    AF = mybir.ActivationFunctionType
    ALU = mybir.AluOpType
    DR = mybir.MatmulPerfMode.DoubleRow

    nc = bacc.Bacc("TRN2", target_bir_lowering=False, debug=False,
                   enable_asserts=enable_asserts, num_devices=N_CORES)
    fm1 = nc.dram_tensor("fm1", [nbatch, C, N], f32, kind="ExternalInput").ap()
    fm2 = nc.dram_tensor("fm2", [nbatch, C, N], f32, kind="ExternalInput").ap()
    o1 = nc.dram_tensor("o1", [nbatch, C, N], f32, kind="ExternalOutput").ap()
    o2 = nc.dram_tensor("o2", [nbatch, C, N], f32, kind="ExternalOutput").ap()

    with tile.TileContext(nc) as tc:
        with (
            tc.tile_pool(name="sb", bufs=2) as sb,
            tc.tile_pool(name="ps", bufs=2, space="PSUM") as ps,
            tc.tile_pool(name="dr", bufs=2, space="DRAM") as dram,
        ):
            ones8 = sb.tile([128, 2, 16], f8, tag="ones8", bufs=1)
            nc.vector.memset(ones8[:], 1.0)

            def newton16(u):
                """[128, 16] f32 sums-of-squares -> rsqrt, 3 Newton iters."""
                yt = sb.tile([128, 16], f32, tag="nwt_y", bufs=2)
                ya = sb.tile([128, 16], f32, tag="nwt_a", bufs=2)
                nc.vector.memset(yt[:], RSQRT_SEED)
                for _ in range(3):
                    nc.vector.tensor_tensor(out=ya[:], in0=yt[:], in1=yt[:],
                                            op=ALU.mult)
                    nc.vector.tensor_tensor(out=ya[:], in0=ya[:], in1=u,
                                            op=ALU.mult)
                    nc.vector.tensor_scalar(
                        out=ya[:], in0=ya[:], scalar1=-0.5, scalar2=1.5,
                        op0=ALU.mult, op1=ALU.add)
                    nc.vector.tensor_tensor(out=yt[:], in0=yt[:], in1=ya[:],
                                            op=ALU.mult)
                return yt

            def col_bcast(colsb, tag):
                """[128, 128] bf16 (cols t<7 = vals for t*128+p) ->
                [128, NP] bf16 broadcast tile via dmaT + gather + bcast."""
                rT = sb.tile([128, 128], bf16, tag=tag + "_T", bufs=2)
                nc.sync.dma_start_transpose(out=rT[:], in_=colsb[:, 0:128])
                d = dram.tile([1, NP], bf16, tag=tag + "_d", bufs=2)
                nc.sync.dma_start(out=d[:].rearrange("a (t x) -> a t x", t=7),
                                  in_=rT[0:7, :])
                return bcast_row(d, tag)

            def bcast_row(d, tag):
                dap = d[:]
                srcap = bass.AP(tensor=dap.tensor, offset=dap.offset,
                                ap=[[0, 128]] + list(dap.ap))
                out = sb.tile([128, d.shape[-1]], bf16, tag=tag + "_B", bufs=1)
                nc.sync.dma_start(
                    out=out[:].rearrange("p (a x) -> p a x", a=1), in_=srcap)
                return out

            # ---------------- phases ----------------
            def load(b):
                f1c = sb.tile([128, 4, N], f32, tag="f1_32", bufs=2)
                f2c = sb.tile([128, 4, N], f32, tag="f2_32", bufs=2)
                for j in range(4):
                    csl = slice(j * 128, (j + 1) * 128)
                    nc.sync.dma_start(out=f1c[:, j, :], in_=fm1[b, csl, :])
                    nc.sync.dma_start(out=f2c[:, j, :], in_=fm2[b, csl, :])
                return f1c, f2c

            def prepA(b, loaded):
                """bf16 staging (+S accums), xbar transposes, fp8 casts."""
                f1c, f2c = loaded
                f1b = sb.tile([128, 4, NP], bf16, tag="f1b", bufs=2)
                f2b = sb.tile([128, 4, NP], bf16, tag="f2b", bufs=2)
                if b < 2:  # zero the n-pad once per pool buffer
                    nc.gpsimd.memset(f1b[:, :, N:], 0)
                    nc.gpsimd.memset(f2b[:, :, N:], 0)
                s1 = sb.tile([128, 4], f32, tag="s1", bufs=2)
                s2 = sb.tile([128, 4], f32, tag="s2", bufs=2)
                with nc.allow_low_precision(reason="bf16 staging"):
                    for j in range(4):
                        nc.scalar.activation(
                            out=f1b[:, j, :N], in_=f1c[:, j, :], func=AF.Copy,
                            accum_out=s1[:, j:j + 1])
                    for j in range(4):
                        nc.scalar.activation(
                            out=f2b[:, j, :N], in_=f2c[:, j, :], func=AF.Copy,
                            accum_out=s2[:, j:j + 1])

                f1bT = sb.tile([128, 7, C], bf16, tag="f1bT", bufs=1)
                f2bT = sb.tile([128, 7, C], bf16, tag="f2bT", bufs=1)
                for j in range(4):
                    nc.sync.dma_start_transpose(
                        out=f1bT[:, 0:7, j * 128:(j + 1) * 128],
                        in_=f1b[:, j, :])
                for j in range(4):
                    nc.sync.dma_start_transpose(
                        out=f2bT[:, 0:7, j * 128:(j + 1) * 128],
                        in_=f2b[:, j, :])

                f1q = sb.tile([128, 4, NP], f8, tag="f1q", bufs=2)
                nc.gpsimd.dma_start(
                    out=f1q[:].rearrange("p a x -> p (a x)"),
                    in_=f1b[:].rearrange("p a x -> p (a x)"))
                f1T = sb.tile([128, 7, C], f8, tag="f1T", bufs=2)
                nc.gpsimd.dma_start(
                    out=f1T[:].rearrange("p a x -> p (a x)"),
                    in_=f1bT[:].rearrange("p a x -> p (a x)"))
                f2T = sb.tile([128, 7, C], f8, tag="f2T", bufs=2)
                nc.gpsimd.dma_start(
                    out=f2T[:].rearrange("p a x -> p (a x)"),
                    in_=f2bT[:].rearrange("p a x -> p (a x)"))
                return f1b, f2b, f1bT, f2bT, f1q, f1T, f2T, s1, s2

            def prepB(b, stA):
                """ssq -> rsqrt -> r2 broadcast -> scaled fp8 f2n."""
                f1b, f2b, f1bT, f2bT, f1q, f1T, f2T, s1, s2 = stA
                ssq = sb.tile([128, 16], f32, tag="ssq", bufs=2)
                nc.vector.memset(ssq[:], 1.0)
                junkv = sb.tile([128, C], bf16, tag="junkv", bufs=1)
                junka = sb.tile([128, C], bf16, tag="junka", bufs=1)
                with nc.allow_low_precision(reason="ssq junk out"):
                    for t in range(7):
                        nc.vector.scalar_tensor_tensor(
                            out=junkv[:], in0=f1bT[:, t, :], scalar=1.0,
                            in1=f1bT[:, t, :], op0=ALU.mult, op1=ALU.mult,
                            accum_out=ssq[:, t:t + 1])
                for t in range(7):
                    nc.scalar.activation(
                        out=junka[:], in_=f2bT[:, t, :], func=AF.Square,
                        accum_out=ssq[:, 8 + t:9 + t])

                y = newton16(ssq[:])
                r1s = sb.tile([128, 8], f32, tag="r1s", bufs=2)
                nc.vector.tensor_scalar(out=r1s[:], in0=y[:, 0:8],
                                        scalar1=0.0625, scalar2=None,
                                        op0=ALU.mult)
                r2cb = sb.tile([128, 128], bf16, tag="r2cb", bufs=2)
                if b < 2:
                    nc.gpsimd.memset(r2cb[:, 8:], 0)
                with nc.allow_low_precision(reason="bf16 r2 cols"):
                    nc.vector.tensor_scalar(out=r2cb[:, 0:8], in0=y[:, 8:16],
                                            scalar1=1.0, scalar2=None,
                                            op0=ALU.mult)
                r2B = col_bcast(r2cb, "r2")

                f2n = sb.tile([128, 4, NP], f8, tag="f2n", bufs=2)
                with nc.allow_low_precision(reason="fp8 scaled f2"):
                    for j in range(4):
                        nc.vector.scalar_tensor_tensor(
                            out=f2n[:, j, :], in0=f2b[:, j, :], scalar=-16.0,
                            in1=r2B[:, :], op0=ALU.mult, op1=ALU.mult)
                return stA + (r1s, f2n)

            def compA(b, st):
                """gram + exp + shift1 + E-transpose + Ep1 cast."""
                f1b, f2b, f1bT, f2bT, f1q, f1T, f2T, s1, s2, r1s, f2n = st
                E1 = sb.tile([128, 7, NP], bf16, tag="E1", bufs=1)
                Ep1b = sb.tile([128, 7, N], bf16, tag="Ep1b", bufs=1)
                E2b = sb.tile([128, 7, NP], bf16, tag="E2b", bufs=1)
                rsc = sb.tile([128, 8], f32, tag="rsc", bufs=2)
                if b < 1:  # n-pad cols, read (as junk) by the E dma transpose
                    nc.gpsimd.memset(E1[:, :, N:], 0)
                nc.vector.memset(rsc[:], 1.0)
                for t in range(7):
                    G = ps.tile([128, 1024], f32, tag="G", bufs=2)
                    nsl = slice(t * 128, (t + 1) * 128)
                    for k in range(2):
                        for hoff, hsz in HALVES:
                            nc.tensor.matmul(
                                G[:, hoff:hoff + hsz],
                                f1q[:, 2 * k:2 * k + 2, nsl],
                                f2n[:, 2 * k:2 * k + 2, hoff:hoff + hsz],
                                start=(k == 0), stop=(k == 1), perf_mode=DR)
                    nc.scalar.activation(
                        out=E1[:, t, :N], in_=G[:, :N], func=AF.Exp,
                        scale=r1s[:, t:t + 1], accum_out=rsc[:, t:t + 1])
                    with nc.allow_low_precision(reason="shifted E1"):
                        nc.vector.tensor_scalar(
                            out=Ep1b[:, t, :], in0=E1[:, t, :N], scalar1=-1.0,
                            scalar2=None, op0=ALU.add)
                    nc.sync.dma_start_transpose(
                        out=E2b[:, 0:7, t * 128:(t + 1) * 128],
                        in_=E1[:, t, :])
                Ep1 = sb.tile([128, 7, N], f8, tag="Ep1", bufs=1)
                nc.gpsimd.dma_start(
                    out=Ep1[:].rearrange("p a x -> p (a x)"),
                    in_=Ep1b[:].rearrange("p a x -> p (a x)"))
                return E2b, Ep1, rsc

            def compB(b, st, cst):
                """shift2 + colsum + output GEMMs + scaling + stores."""
                f1b, f2b, f1bT, f2bT, f1q, f1T, f2T, s1, s2, r1s, f2n = st
                E2b, Ep1, rsc = cst

                # shift2 (4x bf16) then cast to fp8
                Ep2b = sb.tile([128, 7, NP], bf16, tag="Ep2b", bufs=1)
                with nc.allow_low_precision(reason="shifted E2"):
                    for u in range(7):
                        nc.vector.tensor_scalar(
                            out=Ep2b[:, u, :], in0=E2b[:, u, :], scalar1=-1.0,
                            scalar2=None, op0=ALU.add)
                Ep2 = sb.tile([128, 7, NP], f8, tag="Ep2", bufs=1)
                nc.gpsimd.dma_start(
                    out=Ep2[:].rearrange("p a x -> p (a x)"),
                    in_=Ep2b[:].rearrange("p a x -> p (a x)"))

                # cs[m] = colsum(Ep1) via ones-row GEMM; rcB = .001/(cs+N)
                csP = ps.tile([128, 1024], f32, tag="G", bufs=2)
                for hoff, hsz in HALVES:
                    for u in range(3):
                        nc.tensor.matmul(
                            csP[0:1, hoff:hoff + hsz],
                            ones8[:, :, 0:1],
                            Ep1[:, 2 * u:2 * u + 2, hoff:hoff + hsz],
                            start=(u == 0), stop=False, perf_mode=DR)
                    nc.tensor.matmul(
                        csP[0:1, hoff:hoff + hsz],
                        ones8[:, 0, 0:1],
                        Ep1[:, 6, hoff:hoff + hsz],
                        start=False, stop=True)
                csrow = sb.tile([1, N], f32, tag="csrow", bufs=2)
                nc.scalar.activation(out=csrow[:], in_=csP[0:1, :N],
                                     func=AF.Copy, bias=float(N))
                nc.vector.reciprocal(csrow[:], csrow[:])
                crb = sb.tile([1, N], bf16, tag="crb", bufs=2)
                with nc.allow_low_precision(reason="bf16 scale row"):
                    nc.vector.tensor_scalar(out=crb[:], in0=csrow[:],
                                            scalar1=FACTOR, scalar2=None,
                                            op0=ALU.mult)
                d_rc = dram.tile([1, N], bf16, tag="rc_d", bufs=2)
                nc.sync.dma_start(out=d_rc[:], in_=crb[:])
                rcB = bcast_row(d_rc, "rc")

                # rrB = .001/rs via column chain
                rrf = sb.tile([128, 8], f32, tag="rrf", bufs=2)
                nc.vector.reciprocal(rrf[:], rsc[:])
                rrcb = sb.tile([128, 128], bf16, tag="rrcb", bufs=2)
                if b < 2:
                    nc.gpsimd.memset(rrcb[:, 8:], 0)
                with nc.allow_low_precision(reason="bf16 scale cols"):
                    nc.vector.tensor_scalar(out=rrcb[:, 0:8], in0=rrf[:],
                                            scalar1=FACTOR, scalar2=None,
                                            op0=ALU.mult)
                rrB = col_bcast(rrcb, "rr")

                def out_mm(dst, statT, mov, scol, sclB):
                    for ci in range(4):
                        csl = slice(ci * 128, (ci + 1) * 128)
                        P = ps.tile([128, 1024], f32, tag="P", bufs=2)
                        for hoff, hsz in HALVES:
                            for u in range(3):
                                nc.tensor.matmul(
                                    P[:, hoff:hoff + hsz],
                                    statT[:, 2 * u:2 * u + 2, csl],
                                    mov[:, 2 * u:2 * u + 2, hoff:hoff + hsz],
                                    start=(u == 0), stop=False, perf_mode=DR)
                            nc.tensor.matmul(
                                P[:, hoff:hoff + hsz],
                                statT[:, 6, csl],
                                mov[:, 6, hoff:hoff + hsz],
                                start=False, stop=True)
                        O = sb.tile([128, N], f32, tag="O", bufs=2)
                        nc.vector.scalar_tensor_tensor(
                            out=O[:], in0=P[:, :N], scalar=scol[:, ci:ci + 1],
                            in1=sclB[:, :N], op0=ALU.add, op1=ALU.mult)
                        nc.sync.dma_start(out=dst[b, csl, :], in_=O[:])

                out_mm(o2, f1T, Ep1, s1, rcB)
                out_mm(o1, f2T, Ep2, s2, rrB)

            # ---------------- pipeline ----------------
            loads, stA, st = {}, {}, {}
            for j in range(min(3, nbatch)):
                loads[j] = load(j)
            for j in range(min(2, nbatch)):
                stA[j] = prepA(j, loads[j])
            st[0] = prepB(0, stA[0])
            for i in range(nbatch):
                if i + 3 < nbatch:
                    loads[i + 3] = load(i + 3)
                cst = compA(i, st[i])
                if i + 1 < nbatch:
                    st[i + 1] = prepB(i + 1, stA[i + 1])
                compB(i, st[i], cst)
                if i + 2 < nbatch:
                    stA[i + 2] = prepA(i + 2, loads[i + 2])

    nc.compile()
    _BUILT[key] = nc
    return nc


def _run(fm1, fm2, trace=False):
    from concourse.bass_utils import run_bass_kernel_spmd

    fm1 = np.ascontiguousarray(np.asarray(fm1, np.float32).reshape(B_TOTAL, C, N))
    fm2 = np.ascontiguousarray(np.asarray(fm2, np.float32).reshape(B_TOTAL, C, N))
    nc = _build(B_PER_CORE)
    f1s = fm1.reshape(N_CORES, B_PER_CORE, C, N)
    f2s = fm2.reshape(N_CORES, B_PER_CORE, C, N)
    in_maps = [
        {"fm1": np.ascontiguousarray(f1s[i]), "fm2": np.ascontiguousarray(f2s[i])}
        for i in range(N_CORES)
    ]
    res = run_bass_kernel_spmd(nc, in_maps, core_ids=list(range(N_CORES)),
                               trace=trace)
    out1 = np.concatenate([res.results[i]["o1"] for i in range(N_CORES)], axis=0)
    out2 = np.concatenate([res.results[i]["o2"] for i in range(N_CORES)], axis=0)
    out1 = out1.reshape(B_TOTAL, C, 28, 28).astype(np.float32)
    out2 = out2.reshape(B_TOTAL, C, 28, 28).astype(np.float32)
    return (out1, out2), res


def kernel(fm1, fm2):
    (out1, out2), _ = _run(fm1, fm2)
    return out1, out2


# revision 17
# speedup vs baseline: 1.5141x; 1.0134x over previous
# Trainium2 Bass kernel for nn_FDM_3899830304921 (feature-map cosine-sim
# dual-softmax transport), data-parallel over batch on 8 NeuronCores.
#
# v6: PE runs only matmuls (fp8-DR gram + two fp8-DR output GEMMs + tiny
# ones-row GEMMs for colsum and the two sums-of-squares). Transposes ride
# the DMA xbar in bf16 -- one big call per tensor (flat ~1.2us dispatch
# per call), spread across sequencers (scalar/tensor) so the Sync queue
# isn't the bottleneck. rsqrt/reciprocal run as ACT ln/exp row ops (no
# Newton, no 1-partition DVE reciprocal). E-shifts on DVE in bf16 at 4x;
# GPSIMD cast-DMAs produce every fp8 tensor.
#
# Per batch (c=512, n=m=784, pad 896):
#   f1b=bf16(f1)(+S1), f2b=bf16(f2)(+S2)        [ACT copy+accum]
#   f1sq=f1b^2 -> ssq1row = ones^T@f1sq (PE); r1=exp(-.5 ln ssq1)/16 (ACT)
#   f2sq -> ssq2row; r2row=exp(-.5 ln ssq2) -> bcast r2B
#   f1bT=dmaT(f1b) [j,t,k layout], f2bT=dmaT(f2b)
#   f1q,f1T,f2T = fp8 cast-DMAs; f2n = fp8(-16*r2B*f2b) [DVE STT]
#   G=f1q^T@f2n (DR); E1=exp(G*r1cols) (+rowsum rs)
#   Ep1b=E1-1 (4x) -> Ep1 fp8; E2b=dmaT(E1); Ep2b=E2b-1 -> Ep2 fp8
#   cs = ones^T@Ep1 (PE); rcB=exp(-ln(cs+N)+ln .001) bcast
#   rrB = .001/rs (col chain)
#   o2=(f1T^T@Ep1+S1)*rcB; o1=(f2T^T@Ep2+S2)*rrB
import sys

if "/opt/trn_rl_repo" not in sys.path:
    sys.path.insert(0, "/opt/trn_rl_repo")

import math
import numpy as np

B_TOTAL = 32
B_PER_CORE = 4
N_CORES = 8
C = 512
N = 784   # 28*28, both spatial dims
NP = 896  # padded to 7*128 for the DMA xbar transpose
FACTOR = 0.001

HALVES = [(0, 512), (512, 272)]

_BUILT = {}


def _build(nbatch, enable_asserts=False):
    key = (nbatch, enable_asserts)
    if key in _BUILT:
        return _BUILT[key]

    import concourse.bass as bass
    import concourse.tile as tile
    from concourse import bacc, mybir

    f32 = mybir.dt.float32
    f8 = mybir.dt.float8e4
    bf16 = mybir.dt.bfloat16
    AF = mybir.ActivationFunctionType
    ALU = mybir.AluOpType
    DR = mybir.MatmulPerfMode.DoubleRow

    nc = bacc.Bacc("TRN2", target_bir_lowering=False, debug=False,
                   enable_asserts=enable_asserts, num_devices=N_CORES)
    fm1 = nc.dram_tensor("fm1", [nbatch, C, N], f32, kind="ExternalInput").ap()
    fm2 = nc.dram_tensor("fm2", [nbatch, C, N], f32, kind="ExternalInput").ap()
    o1 = nc.dram_tensor("o1", [nbatch, C, N], f32, kind="ExternalOutput").ap()
    o2 = nc.dram_tensor("o2", [nbatch, C, N], f32, kind="ExternalOutput").ap()

    with tile.TileContext(nc) as tc:
        with (
            tc.tile_pool(name="sb", bufs=2) as sb,
            tc.tile_pool(name="ps", bufs=2, space="PSUM") as ps,
            tc.tile_pool(name="dr", bufs=2, space="DRAM") as dram,
        ):
            ones8 = sb.tile([128, 2, 16], f8, tag="ones8", bufs=1)
            nc.vector.memset(ones8[:], 1.0)
            onesb = sb.tile([128, 1], bf16, tag="onesb", bufs=1)
            nc.vector.memset(onesb[:], 1.0)
            bln16 = sb.tile([128, 1], f32, tag="bln16", bufs=1)
            nc.vector.memset(bln16[:], -math.log(16.0))
            blnf = sb.tile([128, 1], f32, tag="blnf", bufs=1)
            nc.vector.memset(blnf[:], math.log(FACTOR))
            bN = sb.tile([128, 1], f32, tag="bN", bufs=1)
            nc.vector.memset(bN[:], float(N))

            def bcast_row(d, tag, eng):
                dap = d[:]
                srcap = bass.AP(tensor=dap.tensor, offset=dap.offset,
                                ap=[[0, 128]] + list(dap.ap))
                out = sb.tile([128, d.shape[-1]], bf16, tag=tag + "_B", bufs=1)
                eng.dma_start(
                    out=out[:].rearrange("p (a x) -> p a x", a=1), in_=srcap)
                return out

            def ssq_row(fsq_src, jtile, tag):
                """ones^T @ (src_j^2) accumulated over j -> [1, N] psum."""
                sP = ps.tile([128, 1024], f32, tag="P", bufs=2)
                for j in range(4):
                    fsq = sb.tile([128, N], bf16, tag="fsq", bufs=2)
                    with nc.allow_low_precision(reason="squares"):
                        nc.vector.tensor_tensor(
                            out=fsq[:], in0=fsq_src[:, j, :N],
                            in1=fsq_src[:, j, :N], op=ALU.mult)
                    for hoff, hsz in HALVES:
                        nc.tensor.matmul(
                            sP[0:1, hoff:hoff + hsz], onesb[:],
                            fsq[:, hoff:hoff + hsz],
                            start=(j == 0), stop=(j == 3))
                return sP

            # ---------------- phases ----------------
            def load(b):
                f1c = sb.tile([128, 4, N], f32, tag="f1_32", bufs=2)
                f2c = sb.tile([128, 4, N], f32, tag="f2_32", bufs=2)
                nc.sync.dma_start(
                    out=f1c[:], in_=fm1[b].rearrange("(j p) n -> p j n", p=128))
                nc.sync.dma_start(
                    out=f2c[:], in_=fm2[b].rearrange("(j p) n -> p j n", p=128))
                return f1c, f2c

            def prepA(b, loaded):
                """bf16 staging (+S accums), xbar transposes, fp8 casts."""
                f1c, f2c = loaded
                f1b = sb.tile([128, 4, NP], bf16, tag="f1b", bufs=2)
                f2b = sb.tile([128, 4, NP], bf16, tag="f2b", bufs=2)
                if b < 2:  # zero the n-pad once per pool buffer
                    nc.gpsimd.memset(f1b[:, :, N:], 0)
                    nc.gpsimd.memset(f2b[:, :, N:], 0)
                s1 = sb.tile([128, 4], f32, tag="s1", bufs=2)
                s2 = sb.tile([128, 4], f32, tag="s2", bufs=2)
                with nc.allow_low_precision(reason="bf16 staging"):
                    for j in range(4):
                        nc.scalar.activation(
                            out=f1b[:, j, :N], in_=f1c[:, j, :], func=AF.Copy,
                            accum_out=s1[:, j:j + 1])
                    for j in range(4):
                        nc.scalar.activation(
                            out=f2b[:, j, :N], in_=f2c[:, j, :], func=AF.Copy,
                            accum_out=s2[:, j:j + 1])

                # [p, j, t, k] layout: c = j*128+k, n = t*128+p
                f1bT = sb.tile([128, 4, 7, 128], bf16, tag="f1bT", bufs=1)
                f2bT = sb.tile([128, 4, 7, 128], bf16, tag="f2bT", bufs=1)
                nc.scalar.dma_start_transpose(
                    out=f1bT[:].rearrange("p j t k -> p (j t) k"),
                    in_=f1b[:].rearrange("p a x -> p (a x)"))
                nc.scalar.dma_start_transpose(
                    out=f2bT[:].rearrange("p j t k -> p (j t) k"),
                    in_=f2b[:].rearrange("p a x -> p (a x)"))

                f1q = sb.tile([128, 4, NP], f8, tag="f1q", bufs=2)
                nc.gpsimd.dma_start(
                    out=f1q[:].rearrange("p a x -> p (a x)"),
                    in_=f1b[:].rearrange("p a x -> p (a x)"))
                f1T = sb.tile([128, 4, 7, 128], f8, tag="f1T", bufs=2)
                nc.gpsimd.dma_start(
                    out=f1T[:].rearrange("p a t k -> p (a t k)"),
                    in_=f1bT[:].rearrange("p a t k -> p (a t k)"))
                f2T = sb.tile([128, 4, 7, 128], f8, tag="f2T", bufs=2)
                nc.gpsimd.dma_start(
                    out=f2T[:].rearrange("p a t k -> p (a t k)"),
                    in_=f2bT[:].rearrange("p a t k -> p (a t k)"))
                return f1b, f2b, f1q, f1T, f2T, s1, s2

            def prepB(b, stA):
                """sums of squares -> r1 cols / r2 bcast -> scaled fp8 f2n."""
                f1b, f2b, f1q, f1T, f2T, s1, s2 = stA
                # r1: rsqrt(ssq1)/16 as per-partition cols via ln/exp + dmaT
                s1P = ssq_row(f1b, 4, "f1sq")
                lnr = sb.tile([1, NP], f32, tag="lnrow", bufs=2)
                nc.scalar.activation(out=lnr[:, :N], in_=s1P[0:1, :N],
                                     func=AF.Ln)
                r1pad = sb.tile([16, NP], bf16, tag="r1pad", bufs=2)
                if b < 2:
                    nc.vector.memset(r1pad[:], 0)
                with nc.allow_low_precision(reason="bf16 r1 row"):
                    nc.scalar.activation(
                        out=r1pad[0:1, :N], in_=lnr[:, :N], func=AF.Exp,
                        scale=-0.5, bias=bln16[0:1, :])
                r1colsb = sb.tile([128, 7, 16], bf16, tag="r1cb", bufs=2)
                nc.scalar.dma_start_transpose(out=r1colsb[:], in_=r1pad[:])
                r1cols = sb.tile([128, 7, 16], f32, tag="r1c", bufs=2)
                nc.vector.tensor_copy(out=r1cols[:], in_=r1colsb[:])

                # r2: rsqrt(ssq2) as a broadcast row
                s2P = ssq_row(f2b, 4, "f2sq")
                lnr2 = sb.tile([1, NP], f32, tag="lnrow", bufs=2)
                nc.scalar.activation(out=lnr2[:, :N], in_=s2P[0:1, :N],
                                     func=AF.Ln)
                r2row = sb.tile([1, N], bf16, tag="rowb", bufs=2)
                with nc.allow_low_precision(reason="bf16 r2 row"):
                    nc.scalar.activation(out=r2row[:], in_=lnr2[:, :N],
                                         func=AF.Exp, scale=-0.5)
                d2 = dram.tile([1, N], bf16, tag="r2_d", bufs=2)
                nc.sync.dma_start(out=d2[:], in_=r2row[:])
                r2B = bcast_row(d2, "r2", nc.sync)

                f2n = sb.tile([128, 4, N], f8, tag="f2n", bufs=2)
                with nc.allow_low_precision(reason="fp8 scaled f2"):
                    for j in range(4):
                        nc.vector.scalar_tensor_tensor(
                            out=f2n[:, j, :], in0=f2b[:, j, :N], scalar=-16.0,
                            in1=r2B[:, :], op0=ALU.mult, op1=ALU.mult)
                return stA + (r1cols, f2n)

            def compA(b, st):
                """gram + exp + shift1 + E-transpose + Ep1 cast."""
                f1b, f2b, f1q, f1T, f2T, s1, s2, r1cols, f2n = st
                E1 = sb.tile([128, 7, NP], bf16, tag="E1", bufs=1)
                Ep1b = sb.tile([128, 7, N], bf16, tag="Ep1b", bufs=1)
                E2b = sb.tile([128, 7, NP], bf16, tag="E2b", bufs=1)
                rsc = sb.tile([128, 8], f32, tag="rsc", bufs=2)
                if b < 1:  # n-pad cols, read (as junk) by the E dma transpose
                    nc.gpsimd.memset(E1[:, :, N:], 0)
                nc.vector.memset(rsc[:], 1.0)
                for t in range(7):
                    G = ps.tile([128, 1024], f32, tag="G", bufs=2)
                    nsl = slice(t * 128, (t + 1) * 128)
                    for k in range(2):
                        for hoff, hsz in HALVES:
                            nc.tensor.matmul(
                                G[:, hoff:hoff + hsz],
                                f1q[:, 2 * k:2 * k + 2, nsl],
                                f2n[:, 2 * k:2 * k + 2, hoff:hoff + hsz],
                                start=(k == 0), stop=(k == 1), perf_mode=DR)
                    nc.scalar.activation(
                        out=E1[:, t, :N], in_=G[:, :N], func=AF.Exp,
                        scale=r1cols[:, t, 0:1], accum_out=rsc[:, t:t + 1])
                    with nc.allow_low_precision(reason="shifted E1"):
                        nc.vector.tensor_scalar(
                            out=Ep1b[:, t, :], in0=E1[:, t, :N], scalar1=-1.0,
                            scalar2=None, op0=ALU.add)
                    teng = nc.sync if t % 2 == 0 else nc.scalar
                    teng.dma_start_transpose(
                        out=E2b[:, 0:7, t * 128:(t + 1) * 128],
                        in_=E1[:, t, :])
                Ep1 = sb.tile([128, 7, N], f8, tag="Ep1", bufs=1)
                nc.gpsimd.dma_start(
                    out=Ep1[:].rearrange("p a x -> p (a x)"),
                    in_=Ep1b[:].rearrange("p a x -> p (a x)"))
                return E2b, Ep1, rsc

            def compB(b, st, cst):
                """shift2 + colsum + output GEMMs + scaling + stores."""
                f1b, f2b, f1q, f1T, f2T, s1, s2, r1cols, f2n = st
                E2b, Ep1, rsc = cst

                Ep2b = sb.tile([128, 7, NP], bf16, tag="Ep2b", bufs=1)
                with nc.allow_low_precision(reason="shifted E2"):
                    for u in range(7):
                        nc.vector.tensor_scalar(
                            out=Ep2b[:, u, :], in0=E2b[:, u, :], scalar1=-1.0,
                            scalar2=None, op0=ALU.add)
                Ep2 = sb.tile([128, 7, NP], f8, tag="Ep2", bufs=1)
                nc.gpsimd.dma_start(
                    out=Ep2[:].rearrange("p a x -> p (a x)"),
                    in_=Ep2b[:].rearrange("p a x -> p (a x)"))

                # cs[m] = colsum(Ep1) via ones-row GEMM; rcB = .001/(cs+N)
                csP = ps.tile([128, 1024], f32, tag="P", bufs=2)
                for hoff, hsz in HALVES:
                    for u in range(3):
                        nc.tensor.matmul(
                            csP[0:1, hoff:hoff + hsz],
                            ones8[:, :, 0:1],
                            Ep1[:, 2 * u:2 * u + 2, hoff:hoff + hsz],
                            start=(u == 0), stop=False, perf_mode=DR)
                    nc.tensor.matmul(
                        csP[0:1, hoff:hoff + hsz],
                        ones8[:, 0, 0:1],
                        Ep1[:, 6, hoff:hoff + hsz],
                        start=False, stop=True)
                lncs = sb.tile([1, NP], f32, tag="lnrow", bufs=2)
                nc.scalar.activation(out=lncs[:, :N], in_=csP[0:1, :N],
                                     func=AF.Ln, bias=bN[0:1, :])
                crb = sb.tile([1, N], bf16, tag="rowb", bufs=2)
                with nc.allow_low_precision(reason="bf16 scale row"):
                    nc.scalar.activation(out=crb[:], in_=lncs[:, :N],
                                         func=AF.Exp, scale=-1.0,
                                         bias=blnf[0:1, :])
                d_rc = dram.tile([1, N], bf16, tag="rc_d", bufs=2)
                nc.sync.dma_start(out=d_rc[:], in_=crb[:])
                rcB = bcast_row(d_rc, "rc", nc.sync)

                # rrB = .001/rs via ln/exp on the accumulated rowsum cols
                lnrs = sb.tile([128, 8], f32, tag="lnrs", bufs=2)
                nc.scalar.activation(out=lnrs[:], in_=rsc[:], func=AF.Ln)
                rrcb = sb.tile([128, 128], bf16, tag="rrcb", bufs=2)
                if b < 2:
                    nc.gpsimd.memset(rrcb[:, 8:], 0)
                with nc.allow_low_precision(reason="bf16 scale cols"):
                    nc.scalar.activation(out=rrcb[:, 0:8], in_=lnrs[:],
                                         func=AF.Exp, scale=-1.0,
                                         bias=blnf[:, :])
                rrT = sb.tile([128, 128], bf16, tag="rr_T", bufs=2)
                nc.sync.dma_start_transpose(out=rrT[:], in_=rrcb[:])
                d_rr = dram.tile([1, NP], bf16, tag="rr_d", bufs=2)
                nc.sync.dma_start(out=d_rr[:].rearrange("a (t x) -> a t x", t=7),
                                  in_=rrT[0:7, :])
                rrB = bcast_row(d_rr, "rr", nc.sync)

                def out_mm(dst, statT, mov, scol, sclB, seng):
                    for ci in range(4):
                        P = ps.tile([128, 1024], f32, tag="P", bufs=2)
                        for hoff, hsz in HALVES:
                            for u in range(3):
                                nc.tensor.matmul(
                                    P[:, hoff:hoff + hsz],
                                    statT[:, ci, 2 * u:2 * u + 2, :],
                                    mov[:, 2 * u:2 * u + 2, hoff:hoff + hsz],
                                    start=(u == 0), stop=False, perf_mode=DR)
                            nc.tensor.matmul(
                                P[:, hoff:hoff + hsz],
                                statT[:, ci, 6, :],
                                mov[:, 6, hoff:hoff + hsz],
                                start=False, stop=True)
                        O = sb.tile([128, N], f32, tag="O", bufs=2)
                        nc.vector.scalar_tensor_tensor(
                            out=O[:], in0=P[:, :N], scalar=scol[:, ci:ci + 1],
                            in1=sclB[:, :N], op0=ALU.add, op1=ALU.mult)
                        seng.dma_start(
                            out=dst[b, ci * 128:(ci + 1) * 128, :], in_=O[:])

                out_mm(o2, f1T, Ep1, s1, rcB, nc.scalar)
                out_mm(o1, f2T, Ep2, s2, rrB, nc.sync)

            # ---------------- pipeline ----------------
            loads, stA, st = {}, {}, {}
            for j in range(min(3, nbatch)):
                loads[j] = load(j)
            for j in range(min(2, nbatch)):
                stA[j] = prepA(j, loads[j])
            st[0] = prepB(0, stA[0])
            for i in range(nbatch):
                if i + 3 < nbatch:
                    loads[i + 3] = load(i + 3)
                cst = compA(i, st[i])
                if i + 1 < nbatch:
                    st[i + 1] = prepB(i + 1, stA[i + 1])
                compB(i, st[i], cst)
                if i + 2 < nbatch:
                    stA[i + 2] = prepA(i + 2, loads[i + 2])

    nc.compile()
    _BUILT[key] = nc
    return nc


def _run(fm1, fm2, trace=False):
    from concourse.bass_utils import run_bass_kernel_spmd

    fm1 = np.ascontiguousarray(np.asarray(fm1, np.float32).reshape(B_TOTAL, C, N))
    fm2 = np.ascontiguousarray(np.asarray(fm2, np.float32).reshape(B_TOTAL, C, N))
    nc = _build(B_PER_CORE)
    f1s = fm1.reshape(N_CORES, B_PER_CORE, C, N)
    f2s = fm2.reshape(N_CORES, B_PER_CORE, C, N)
    in_maps = [
        {"fm1": np.ascontiguousarray(f1s[i]), "fm2": np.ascontiguousarray(f2s[i])}
        for i in range(N_CORES)
    ]
    res = run_bass_kernel_spmd(nc, in_maps, core_ids=list(range(N_CORES)),
                               trace=trace)
    out1 = np.concatenate([res.results[i]["o1"] for i in range(N_CORES)], axis=0)
    out2 = np.concatenate([res.results[i]["o2"] for i in range(N_CORES)], axis=0)
    out1 = out1.reshape(B_TOTAL, C, 28, 28).astype(np.float32)
    out2 = out2.reshape(B_TOTAL, C, 28, 28).astype(np.float32)
    return (out1, out2), res


def kernel(fm1, fm2):
    (out1, out2), _ = _run(fm1, fm2)
    return out1, out2


# revision 20
# speedup vs baseline: 2.1010x; 1.3876x over previous
# Trainium2 Bass kernel for nn_FDM_3899830304921 (feature-map cosine-sim
# dual-softmax transport), data-parallel over batch on 8 NeuronCores.
#
# v8: baseline v2.1 architecture (PE transposes, fp8 DoubleRow GEMMs)
# plus: (1) the colsum matmul block is gone -- colsum rides free on the
# accumulator of the ET evacuation copies; (2) the E-shift runs on DVE in
# bf16 at 4x mode and a GPSIMD cast-DMA produces the fp8 copy; (3) the
# per-batch work is split h1a/h2/h1b and interleaved so the PE always has
# ready work queued (no >3.4us idle gaps -> HAM stays at full clock).
#
# Math per batch (c=512, n=m=784):
#   f1q  = fp8(f1)            [c,n]  (+ S1[c]=sum_n f1 via accum, fp32)
#   f1T  = fp8(T(f1q))        [n,c]  (+ ssq1[n] via fp8 squares)
#   f2T  = fp8(T(f2))         [m,c]  (+ ssq2[m] via f32 squares)
#   r1=1/sqrt(ssq1), r2=1/sqrt(ssq2)   (Newton on DVE)
#   f2n  = fp8(-16*r2[m]*f2)  [c,m]
#   G'   = f1q^T @ f2n        [n,m]
#   E    = exp(G'*r1/16) bf16, rowsum rs via accum
#   Epb  = E - 1 (bf16, 4x);  Ep = fp8(Epb) via cast-DMA
#   ET   = T(Ep) fp8, ACT copies w/ accum -> colsum cs
#   S2[c]= sum_m f2  (exact fp32)
#   o2   = (f1T^T @ Ep + S1) * (.001/(cs+N))
#   o1   = (f2T^T @ ET + S2) * (.001/rs)
import sys

if "/opt/trn_rl_repo" not in sys.path:
    sys.path.insert(0, "/opt/trn_rl_repo")

import numpy as np

B_TOTAL = 32
B_PER_CORE = 4
N_CORES = 8
C = 512
N = 784  # 28*28, both spatial dims
FACTOR = 0.001
RSQRT_SEED = 0.044194173824159216  # 1/sqrt(512)

# n (and m) tiling: 6 tiles of 128 + one of 16
NT = [(0, 128), (128, 128), (256, 128), (384, 128), (512, 128), (640, 128), (768, 16)]
# free-dim split of 784 into PSUM-bank-sized pieces
HALVES = [(0, 512), (512, 272)]

_BUILT = {}


def _build(nbatch, enable_asserts=False):
    key = (nbatch, enable_asserts)
    if key in _BUILT:
        return _BUILT[key]

    import concourse.bass as bass
    import concourse.tile as tile
    from concourse import bacc, mybir
    from concourse.masks import make_identity

    f32 = mybir.dt.float32
    f32r = mybir.dt.float32r
    f8 = mybir.dt.float8e4
    bf16 = mybir.dt.bfloat16
    AF = mybir.ActivationFunctionType
    ALU = mybir.AluOpType
    DR = mybir.MatmulPerfMode.DoubleRow

    nc = bacc.Bacc("TRN2", target_bir_lowering=False, debug=False,
                   enable_asserts=enable_asserts, num_devices=N_CORES)
    fm1 = nc.dram_tensor("fm1", [nbatch, C, N], f32, kind="ExternalInput").ap()
    fm2 = nc.dram_tensor("fm2", [nbatch, C, N], f32, kind="ExternalInput").ap()
    o1 = nc.dram_tensor("o1", [nbatch, C, N], f32, kind="ExternalOutput").ap()
    o2 = nc.dram_tensor("o2", [nbatch, C, N], f32, kind="ExternalOutput").ap()

    with tile.TileContext(nc) as tc:
        with (
            tc.tile_pool(name="sb", bufs=2) as sb,
            tc.tile_pool(name="ps", bufs=2, space="PSUM") as ps,
            tc.tile_pool(name="dr", bufs=2, space="DRAM") as dram,
        ):
            identf = sb.tile([128, 128], f32, tag="identf", bufs=1)
            make_identity(nc, identf[:])
            ident8 = sb.tile([128, 128], f8, tag="ident8", bufs=1)
            nc.scalar.copy(ident8[:], identf[:])
            identb = sb.tile([128, 128], bf16, tag="identb", bufs=1)
            nc.gpsimd.tensor_copy(out=identb[:], in_=identf[:])
            identr = sb.tile([128, 128], f32r, tag="identr", bufs=1)
            nc.scalar.copy(identr[:], identf[:])

            def f8ps(ptf, col0, ncols, rows=128):
                a = ptf[:].bitcast(f8)
                return bass.AP(tensor=a.tensor, offset=a.offset + 2 * col0,
                               ap=[list(a.ap)[0], [2, ncols]])[:rows]

            def col_to_row(coltile, eng):
                """[128, 8] bf16 cols -> [1, N] bf16 SBUF row."""
                prt = ps.tile([128, N], f32, tag="big", bufs=4)
                pr = prt[:].bitcast(bf16)
                for t, (noff, nsz) in enumerate(NT):
                    nc.tensor.transpose(
                        pr[:1, noff:noff + nsz],
                        coltile[:nsz, t:t + 1],
                        identb[:nsz, :nsz])
                row = sb.tile([1, N], bf16, tag="row", bufs=3)
                if eng == "v":
                    nc.vector.tensor_copy(out=row[:1, :], in_=pr[:1, :N])
                else:
                    nc.scalar.copy(row[:1, :], pr[:1, :N])
                return row

            def colrecip_bcast(colsum, extra, rowtag, eng="s"):
                """cols [128, 8] f32 sums -> bcast of 0.001/(sum+extra)."""
                rcf = sb.tile([128, 8], f32, tag=rowtag + "_f", bufs=2)
                rcb = sb.tile([128, 8], bf16, tag=rowtag + "_c", bufs=2)
                with nc.allow_low_precision(reason="softmax scale rows"):
                    nc.vector.tensor_scalar(
                        out=rcf[:], in0=colsum, scalar1=1000.0,
                        scalar2=1000.0 * extra, op0=ALU.mult, op1=ALU.add)
                    nc.vector.reciprocal(rcf[:], rcf[:])
                    nc.vector.tensor_scalar(
                        out=rcb[:], in0=rcf[:], scalar1=1.0, scalar2=None,
                        op0=ALU.mult)
                row = col_to_row(rcb, eng)
                d = dram.tile([1, N], bf16, tag=rowtag + "_d", bufs=2)
                nc.sync.dma_start(out=d[:], in_=row[:1, :])
                dap = d[:]
                srcap = bass.AP(tensor=dap.tensor, offset=dap.offset,
                                ap=[[0, 128]] + list(dap.ap))
                out = sb.tile([128, N], bf16, tag=rowtag + "_B", bufs=2)
                nc.sync.dma_start(
                    out=out[:].rearrange("p (a x) -> p a x", a=1), in_=srcap)
                return out

            def newton(ssq1, dst_lo, dst_hi):
                yt = sb.tile([128, 8], f32, tag=f"y{dst_lo}", bufs=2)
                ya = sb.tile([128, 8], f32, tag=f"ya{dst_lo}", bufs=2)
                nc.vector.memset(yt[:], RSQRT_SEED)
                u = ssq1[:, dst_lo:dst_hi]
                for it in range(3):
                    nc.vector.tensor_tensor(out=ya[:], in0=yt[:], in1=yt[:],
                                            op=ALU.mult)
                    nc.vector.tensor_tensor(out=ya[:], in0=ya[:], in1=u,
                                            op=ALU.mult)
                    nc.vector.tensor_scalar(
                        out=ya[:], in0=ya[:], scalar1=-0.5, scalar2=1.5,
                        op0=ALU.mult, op1=ALU.add)
                    nc.vector.tensor_tensor(out=yt[:], in0=yt[:], in1=ya[:],
                                            op=ALU.mult)
                return yt

            def h1a(b):
                """load; quantize f1; T(f2), T(f1q); norms; r2 broadcast."""
                f1_sb = sb.tile([128, 4, N], f32, tag="f1", bufs=2)
                nc.sync.dma_start(
                    out=f1_sb[:],
                    in_=fm1[b].rearrange("(t p) n -> p t n", p=128))
                f2_sb = sb.tile([128, 4, N], f32r, tag="f2", bufs=2)
                nc.sync.dma_start(
                    out=f2_sb[:],
                    in_=fm2[b].rearrange("(t p) n -> p t n", p=128).bitcast(f32r))

                # quantize f1 -> fp8 + exact S1 accum
                f1q = sb.tile([128, 4, N], f8, tag="f1q", bufs=2)
                s1 = sb.tile([128, 4], f32, tag="s1", bufs=2)
                for j in range(4):
                    nc.vector.tensor_scalar(
                        out=f1q[:, j, :], in0=f1_sb[:, j, :],
                        scalar1=1.0, scalar2=0.0, op0=ALU.mult, op1=ALU.add,
                        accum_out=s1[:, j:j + 1])

                # T(f2) f32r; f2T fp8 copies (DVE); ssq2 squares (ACT)
                ssq1 = sb.tile([128, 16], f32, tag="ssq1", bufs=2)
                nc.vector.memset(ssq1[:], 1.0)
                f2T = sb.tile([128, 8, C], f8, tag="f2T", bufs=2)
                if b < 2:
                    nc.gpsimd.memset(f2T[:, 6:8, :], 0)
                junkv = sb.tile([128, C], bf16, tag="junkv", bufs=2)
                for t, (noff, nsz) in enumerate(NT):
                    ptf = ps.tile([128, N], f32, tag="big", bufs=4)
                    pt = ptf[:, :C]
                    for j in range(4):
                        nc.tensor.transpose(
                            pt[:nsz, j * 128:(j + 1) * 128].bitcast(f32r),
                            f2_sb[:, j, noff:noff + nsz],
                            identr[:, :])
                    nc.vector.tensor_copy(out=f2T[:nsz, t, :], in_=pt[:nsz, :])
                    nc.scalar.activation(
                        out=junkv[:nsz], in_=pt[:nsz, :], func=AF.Square,
                        accum_out=ssq1[:nsz, 8 + t:9 + t])

                # T(f1q) fp8 -> f1T; ssq1 squares (ACT)
                f1T = sb.tile([128, 8, C], f8, tag="f1T", bufs=2)
                if b < 2:
                    nc.gpsimd.memset(f1T[:, 6:8, :], 0)
                junk = sb.tile([128, C], bf16, tag="junk", bufs=2)
                for t, (noff, nsz) in enumerate(NT):
                    ptf = ps.tile([128, N], f32, tag="big", bufs=4)
                    for j in range(4):
                        nc.tensor.transpose(
                            f8ps(ptf, j * 128, 128, nsz),
                            f1q[:, j, noff:noff + nsz],
                            ident8[:, :])
                    nc.vector.tensor_copy(out=f1T[:nsz, t, :],
                                          in_=f8ps(ptf, 0, C, nsz))
                    nc.scalar.activation(
                        out=junk[:nsz], in_=f8ps(ptf, 0, C, nsz),
                        func=AF.Square, accum_out=ssq1[:nsz, t:t + 1])

                # r2 chain: newton -> bf16 cols -> row -> DRAM bcast
                y2 = newton(ssq1, 8, 16)
                r2b = sb.tile([128, 8], bf16, tag="r2b", bufs=2)
                with nc.allow_low_precision(reason="bf16 r2 row"):
                    nc.vector.tensor_scalar(
                        out=r2b[:], in0=y2[:], scalar1=1.0, scalar2=None,
                        op0=ALU.mult)
                r2row = col_to_row(r2b, "s")
                d2 = dram.tile([1, N], bf16, tag="r2d", bufs=2)
                nc.sync.dma_start(out=d2[:], in_=r2row[:1, :])
                dap = d2[:]
                srcap = bass.AP(tensor=dap.tensor, offset=dap.offset,
                                ap=[[0, 128]] + list(dap.ap))
                r2B = sb.tile([128, N], bf16, tag="r2B", bufs=2)
                nc.sync.dma_start(
                    out=r2B[:].rearrange("p (a x) -> p a x", a=1), in_=srcap)

                # S2[c] = sum_m f2 (exact, ACT accum; junk out) -- emitted
                # after the r2 chain so ACT prioritizes the ssq2 squares
                s2 = sb.tile([128, 4], f32, tag="s2", bufs=2)
                junkb = sb.tile([128, N], bf16, tag="junkb", bufs=2)
                for j in range(4):
                    nc.scalar.activation(
                        out=junkb[:], in_=f2_sb[:, j, :].bitcast(f32),
                        func=AF.Copy, accum_out=s2[:, j:j + 1])

                # r1 chain (needed first at exp t0)
                y1 = newton(ssq1, 0, 8)
                r1s = sb.tile([128, 8], f32, tag="r1s", bufs=2)
                nc.vector.tensor_scalar(
                    out=r1s[:], in0=y1[:], scalar1=0.0625, scalar2=None,
                    op0=ALU.mult)
                return f1q, f2_sb, f1T, f2T, s1, s2, r2B, r1s

            def h1b(b, stA):
                """f2n quantize; gram; exp; bf16 shift + fp8 cast."""
                f1q, f2_sb, f1T, f2T, s1, s2, r2B, r1s = stA
                f2n = sb.tile([128, 4, N], f8, tag="f2n", bufs=2)
                with nc.allow_low_precision(reason="fp8 scaled f2"):
                    for j in range(4):
                        nc.vector.scalar_tensor_tensor(
                            out=f2n[:, j, :], in0=f2_sb[:, j, :].bitcast(f32),
                            scalar=-16.0, in1=r2B[:, :], op0=ALU.mult,
                            op1=ALU.mult)

                E = sb.tile([128, 7, N], bf16, tag="E", bufs=2)
                Epb = sb.tile([128, 7, N], bf16, tag="Epb", bufs=1)
                if b < 1:  # rows 16.. of the 16-tall tail tile stay zero
                    nc.gpsimd.memset(Epb[:, 6, :], 0)
                rsc = sb.tile([128, 8], f32, tag="rsc", bufs=2)
                nc.vector.memset(rsc[:], 1.0)
                for t, (noff, nsz) in enumerate(NT):
                    G = ps.tile([128, N], f32, tag="big", bufs=4)
                    for k in range(2):
                        for hoff, hsz in HALVES:
                            nc.tensor.matmul(
                                G[:nsz, hoff:hoff + hsz],
                                f1q[:, 2 * k:2 * k + 2, noff:noff + nsz],
                                f2n[:, 2 * k:2 * k + 2, hoff:hoff + hsz],
                                start=(k == 0), stop=(k == 1), perf_mode=DR)
                    nc.scalar.activation(
                        out=E[:nsz, t, :], in_=G[:nsz, :], func=AF.Exp,
                        scale=r1s[:nsz, t:t + 1],
                        accum_out=rsc[:nsz, t:t + 1])
                    with nc.allow_low_precision(reason="shifted E"):
                        nc.vector.tensor_scalar(
                            out=Epb[:nsz, t, :], in0=E[:nsz, t, :],
                            scalar1=-1.0, scalar2=None, op0=ALU.add)

                # fp8 copy of the shifted E via GPSIMD cast-DMA
                Ep = sb.tile([128, 8, N], f8, tag="Ep", bufs=2)
                if b < 2:
                    nc.gpsimd.memset(Ep[:, 7, :], 0)
                nc.gpsimd.dma_start(
                    out=Ep[:, 0:7, :].rearrange("p a x -> p (a x)"),
                    in_=Epb[:].rearrange("p a x -> p (a x)"))
                return Ep, rsc

            def h2(b, stA, stB):
                f1q, f2_sb, f1T, f2T, s1, s2, r2B, r1s = stA
                Ep, rsc = stB

                # ET = T(Ep) fp8; ACT evac copies carry the colsum accum
                ET = sb.tile([128, 8, N], f8, tag="ET", bufs=2)
                if b < 2:
                    nc.gpsimd.memset(ET[:, 6:8, :], 0)
                csc = sb.tile([128, 8], f32, tag="csc", bufs=2)
                nc.vector.memset(csc[:], 1.0)
                for t, (moff, msz) in enumerate(NT):
                    pmf = ps.tile([128, N], f32, tag="big", bufs=4)
                    for u, (noff, nsz) in enumerate(NT):
                        nc.tensor.transpose(
                            f8ps(pmf, noff, nsz, msz),
                            Ep[:nsz, u, moff:moff + msz],
                            ident8[:nsz, :nsz])
                    nc.scalar.activation(
                        out=ET[:msz, t, :], in_=f8ps(pmf, 0, N, msz),
                        func=AF.Copy, accum_out=csc[:msz, t:t + 1])

                def out_mm(dst, statT, mov, scol, sclB):
                    """dst[b, c, :] = (statT^T @ mov + scol) * sclB."""
                    for ci in range(4):
                        csl = slice(ci * 128, (ci + 1) * 128)
                        P = ps.tile([128, N], f32, tag="big", bufs=4)
                        for u in range(4):
                            for hoff, hsz in HALVES:
                                nc.tensor.matmul(
                                    P[:, hoff:hoff + hsz],
                                    statT[:, 2 * u:2 * u + 2, csl],
                                    mov[:, 2 * u:2 * u + 2, hoff:hoff + hsz],
                                    start=(u == 0), stop=(u == 3), perf_mode=DR)
                        O = sb.tile([128, N], f32, tag="o", bufs=4)
                        nc.vector.scalar_tensor_tensor(
                            out=O[:], in0=P[:], scalar=scol[:, ci:ci + 1],
                            in1=sclB[:, :], op0=ALU.add, op1=ALU.mult)
                        nc.sync.dma_start(out=dst[b, csl, :], in_=O[:])

                # out2 mms start as soon as Ep is cast; their finals wait on
                # rcB, which resolves while the mms run
                rcB = colrecip_bcast(csc[:, 0:8], float(N), "rc")
                out_mm(o2, f1T, Ep, s1, rcB)
                rrB = colrecip_bcast(rsc[:, 0:8], 0.0, "rr")
                out_mm(o1, f2T, ET, s2, rrB)

            # pipeline: h1a runs one batch ahead of h2, h1b between them
            stA = {0: h1a(0)}
            stB = {0: h1b(0, stA[0])}
            for b in range(nbatch):
                if b + 1 < nbatch:
                    stA[b + 1] = h1a(b + 1)
                h2(b, stA[b], stB[b])
                if b + 1 < nbatch:
                    stB[b + 1] = h1b(b + 1, stA[b + 1])

    nc.compile()
    _BUILT[key] = nc
    return nc


def _run(fm1, fm2, trace=False):
    from concourse.bass_utils import run_bass_kernel_spmd

    fm1 = np.ascontiguousarray(np.asarray(fm1, np.float32).reshape(B_TOTAL, C, N))
    fm2 = np.ascontiguousarray(np.asarray(fm2, np.float32).reshape(B_TOTAL, C, N))
    nc = _build(B_PER_CORE)
    f1s = fm1.reshape(N_CORES, B_PER_CORE, C, N)
    f2s = fm2.reshape(N_CORES, B_PER_CORE, C, N)
    in_maps = [
        {"fm1": np.ascontiguousarray(f1s[i]), "fm2": np.ascontiguousarray(f2s[i])}
        for i in range(N_CORES)
    ]
    res = run_bass_kernel_spmd(nc, in_maps, core_ids=list(range(N_CORES)),
                               trace=trace)
    out1 = np.concatenate([res.results[i]["o1"] for i in range(N_CORES)], axis=0)
    out2 = np.concatenate([res.results[i]["o2"] for i in range(N_CORES)], axis=0)
    out1 = out1.reshape(B_TOTAL, C, 28, 28).astype(np.float32)
    out2 = out2.reshape(B_TOTAL, C, 28, 28).astype(np.float32)
    return (out1, out2), res


def kernel(fm1, fm2):
    (out1, out2), _ = _run(fm1, fm2)
    return out1, out2
